# revision 7
# baseline (speedup 1.0000x reference)
"""Trainium2 Bass kernel for the ConvNet problem.

Per 512-sample subtile (feature-major via host-side transpose):
  windowed sums s=A@x/sqrt(10), s2=A@x^2 (banded matmuls) -> std =
  sqrt((s2-s^2)/9) -> conv1 (dense 120->190 as 3 overlapping 70-row
  chunks) -> conv2 (3 single-K matmuls, 120 outs each) -> fc1 -> fc2 ->
  pamap (weight-streaming, fp16 weights) -> batched log_softmax tail.

All matmul operands are float32r (full-rate PE, fp32 bits).  Biases are
folded into the matmuls via ones-rows appended to the activations, so
every epilogue is a single merged relu over a multi-bank PSUM tile.
Sharding: pure data parallelism, batch split 8 ways across NeuronCores.
"""

import math
import os

import numpy as np

# ---------------------------------------------------------------------------
# Problem constants (hardcoded; kernel.py must be self-contained)
# ---------------------------------------------------------------------------
B_TOTAL, L, C = 131072, 50, 3
WIN = 10
NCORES = 8
B_CORE = B_TOTAL // NCORES          # 16384
NSUB = 512                          # samples per subtile (matmul N)
NST_FULL = B_CORE // NSUB           # 32 subtiles per core
F_IN = L * C                        # 150
F_USE = 147                         # features consumed (x[49,:] unused)
F_STD = 120                         # 40 windows x 3 channels
F_C1 = 190                          # 38 x 5
F_C2 = 360                          # 36 x 10
F_FC2 = 64
F_OUT = 12

# conv1 emitted as 3 overlapping 70-row chunks; conv2 out chunks of 120
C1_LO = (0, 60, 120)
C1_H = 70                           # rows per conv1 chunk (+1 ones row)
C2_M = 120                          # conv2 outs per chunk (3 chunks = 360)

# debug knob (harness never sets this; default = full problem)
_NST = int(os.environ.get("ATRN_NST", str(NST_FULL)))


# ---------------------------------------------------------------------------
# Tile drain patch: walrus in this container rejects >2 sem waits on a
# CTRL-class (Drain) instruction.  Spread the end-of-kernel global-clock waits
# across per-proc SP nops (one sem each) before an unadorned drain.
# ---------------------------------------------------------------------------
def _install_drain_patch():
    import concourse.tile as tile
    from concourse.tile_scheduler import N_PROCS
    from concourse.vector_clock import ScopedClock, VectorClock

    if getattr(tile.TileContext, "_drain_patch_installed", False):
        return

    def _patched_drain_and_barrier(self, tick_clock, wait_clock):
        nc = self.nc
        gc = tick_clock.global_clock
        for p in range(N_PROCS):
            if gc[p] <= 0:
                continue
            v = [0] * N_PROCS
            v[p] = gc[p]
            nop = nc.sync.nop()
            wait_clock.add_sem_waits(nop.ins, ScopedClock({None: VectorClock(v)}))
        nc.sync.drain()
        nc.all_engine_barrier()
        assert self.sems is not None
        popped = nc._tile_sem_poison_stack.pop()
        assert popped is self._sem_poison
        nc.clear_and_free_semaphores(list(self.sems.allocated().values()))
        nc.all_engine_barrier()

    tile.TileContext._drain_and_barrier = _patched_drain_and_barrier
    tile.TileContext._drain_patch_installed = True


def _split_excess_waits(nc):
    """Hoist excess sem waits onto same-engine nops inserted just before."""
    from concourse import mybir

    ctr = 0
    for f in nc.m.functions:
        for blk in f.blocks:
            il = blk.instructions
            i = 0
            while i < len(il):
                ins = il[i]
                si = ins.sync_info
                cap = 1  # this walrus build rejects >1 sem wait per engine op
                if si is not None and len(si.on_wait) > cap:
                    waits = list(si.on_wait)
                    extra, keep = waits[:-cap], waits[-cap:]
                    for w in extra:
                        ctr += 1
                        nop = mybir.InstNoOp(name=f"waitsplit-{ctr}",
                                             ins=[], outs=[])
                        nop.engine = ins.engine
                        nop.sync_info = type(si)(on_wait=[w], on_update=[])
                        nc.register_instruction(nop, overwrite=True)
                        il.insert(i, nop)
                        i += 1
                    ins.sync_info = type(si)(on_wait=keep,
                                             on_update=list(si.on_update))
                i += 1


# ---------------------------------------------------------------------------
# Host-side weight preprocessing
# ---------------------------------------------------------------------------
class _BlobLayout:
    def __init__(self):
        self.cols = 0
        self.slots = {}

    def add(self, name, rows, cols):
        self.slots[name] = (self.cols, rows, cols)
        self.cols += cols
        return self.slots[name]


_LAY = _BlobLayout()
_LAY.add("sA_a", 128, F_STD)
_LAY.add("sA_b", 19, F_STD)
_LAY.add("s2_a", 128, F_STD)
_LAY.add("s2_b", 19, F_STD)
for m in range(3):
    _LAY.add(f"c1_{m}", F_STD + 1, C1_H)         # +bias row (std ones row)
for m in range(3):
    _LAY.add(f"c2_{m}", C1_H + 1, C2_M)          # +bias row (h1 ones row)
for k in range(3):
    for m in range(2):
        _LAY.add(f"f1_{k}_{m}", C2_M + 1, 128)   # +bias row (k==0 only)
for k in range(2):
    _LAY.add(f"f2_{k}", 128, F_FC2)
_LAY.add("ones", 1, 3 * NSUB)
WF = _LAY.cols


def _build_blob(conv1_w, conv1_b, conv2_w, conv2_b, fc1_w, fc1_b, fc2_w, fc2_b,
                pamap_w, pamap_b):
    blob = np.zeros((128, WF), np.float32)
    bias4 = np.zeros((F_FC2, 1), np.float32)
    bias4[:, 0] = fc2_b.astype(np.float32)

    def put(name, arr):
        off, rows, cols = _LAY.slots[name]
        assert arr.shape == (rows, cols), (name, arr.shape, (rows, cols))
        blob[:rows, off:off + cols] = arr

    # windowed sums: s[3w+c] = (1/sqrt(10)) * sum_k x[3(w+k)+c]
    A = np.zeros((F_USE, F_STD), np.float32)
    for m in range(F_STD):
        w, c = divmod(m, 3)
        for k in range(WIN):
            A[3 * (w + k) + c, m] = 1.0
    put("ones", np.ones((1, 3 * NSUB), np.float32))
    put("sA_a", A[:128] / math.sqrt(10.0))
    put("sA_b", A[128:] / math.sqrt(10.0))
    put("s2_a", A[:128])
    put("s2_b", A[128:])

    # conv1 dense [120 -> 190] + bias row; 3 overlapping 70-row out chunks
    M1 = np.zeros((F_STD + 1, F_C1), np.float32)
    for t in range(38):
        for o in range(5):
            M1[F_STD, 5 * t + o] = conv1_b[o]
            for k in range(3):
                for i in range(3):
                    M1[3 * (t + k) + i, 5 * t + o] = conv1_w[o, i, k]
    for m in range(3):
        put(f"c1_{m}", M1[:, C1_LO[m]:C1_LO[m] + C1_H])

    # conv2 dense [190 -> 360] + bias row; chunk m reads h1 rows
    # C1_LO[m]..C1_LO[m]+69 and writes outs 120m..120m+119
    M2 = np.zeros((F_C1, F_C2), np.float32)
    for t in range(36):
        for o in range(10):
            for k in range(3):
                for i in range(5):
                    M2[5 * (t + k) + i, 10 * t + o] = conv2_w[o, i, k]
    b2 = np.zeros(F_C2, np.float32)
    for t in range(36):
        for o in range(10):
            b2[10 * t + o] = conv2_b[o]
    for m in range(3):
        sub = np.zeros((C1_H + 1, C2_M), np.float32)
        sub[:C1_H] = M2[C1_LO[m]:C1_LO[m] + C1_H, C2_M * m:C2_M * (m + 1)]
        sub[C1_H] = b2[C2_M * m:C2_M * (m + 1)]
        put(f"c2_{m}", sub)

    # fc1 [360 -> 256]; K chunks = conv2 out chunks (120 each + ones row)
    F1 = fc1_w.T.astype(np.float32)          # [360, 256]
    for k in range(3):
        for m in range(2):
            sub = np.zeros((C2_M + 1, 128), np.float32)
            sub[:C2_M] = F1[C2_M * k:C2_M * (k + 1), 128 * m:128 * (m + 1)]
            if k == 0:
                sub[C2_M] = fc1_b[128 * m:128 * (m + 1)]
            put(f"f1_{k}_{m}", sub)
    F2 = fc2_w.T.astype(np.float32)          # [256, 64]
    for k in range(2):
        put(f"f2_{k}", F2[k * 128:(k + 1) * 128])

    # pamap weights + bias row (h4 ones row), fp16
    wp16 = np.zeros((F_FC2 + 1, F_OUT), np.float16)
    wp16[:F_FC2] = pamap_w.T.astype(np.float16)
    wp16[F_FC2] = pamap_b.astype(np.float16)
    return blob, bias4, wp16


def _prep_core_inputs(signal, blob, bias4, wp16):
    """Split + feature-major transpose the signal; one input map per core."""
    nst = _NST
    b_core = nst * NSUB
    sig = np.asarray(signal, np.float32).reshape(-1, F_IN)
    maps = []
    for c in range(NCORES):
        s = sig[c * b_core:(c + 1) * b_core]            # [b_core, 150]
        s3 = s.reshape(nst, NSUB, F_IN)                 # [nst, 512, 150]
        sigA = np.ascontiguousarray(s3[:, :, 0:128].transpose(0, 2, 1))
        sigB = np.ascontiguousarray(s3[:, :, 128:F_USE].transpose(0, 2, 1))
        maps.append({"sigA": sigA, "sigB": sigB, "wb": blob,
                     "wb4": bias4, "wp16": wp16,
                     "ones16": np.ones((1, NSUB), np.float16)})
    return maps


# ---------------------------------------------------------------------------
# Bass program
# ---------------------------------------------------------------------------
_PROGRAM = None


def _build_program(nst):
    import contextlib

    import concourse.bass as bass
    import concourse.tile as tile
    from concourse import mybir

    _install_drain_patch()
    f32 = mybir.dt.float32
    f32r = mybir.dt.float32r
    f16 = mybir.dt.float16
    AF = mybir.ActivationFunctionType
    ALU = mybir.AluOpType

    nc = bass.Bass("TRN2", target_bir_lowering=False, debug=False,
                   num_devices=NCORES)
    sigA = nc.dram_tensor("sigA", [nst, 128, NSUB], f32r, kind="ExternalInput")
    sigB = nc.dram_tensor("sigB", [nst, 19, NSUB], f32r, kind="ExternalInput")
    wb = nc.dram_tensor("wb", [128, WF], f32r, kind="ExternalInput")
    wp16d = nc.dram_tensor("wp16", [F_FC2 + 1, F_OUT], f16,
                           kind="ExternalInput")
    wb4 = nc.dram_tensor("wb4", [F_FC2, 1], f32, kind="ExternalInput")
    ones16d = nc.dram_tensor("ones16", [1, NSUB], f16, kind="ExternalInput")
    out = nc.dram_tensor("out", [nst * NSUB, F_OUT], f32,
                         kind="ExternalOutput")
    # sample index = q*512 + j*128 + p  ->  [half][p][i][j][o]
    out_v = out.rearrange("(h i j p) o -> h p i j o", h=2, j=4, p=128)

    def _w(weights, name):
        off, rows, cols = _LAY.slots[name]
        return weights[0:rows, off:off + cols]

    with tile.TileContext(nc) as tc:
        with contextlib.ExitStack() as ctx:
            singles = ctx.enter_context(tc.tile_pool(name="singles", bufs=1))
            xa_p = ctx.enter_context(tc.tile_pool(name="xa", bufs=3))
            xb_p = ctx.enter_context(tc.tile_pool(name="xb", bufs=3))
            x2_p = ctx.enter_context(tc.tile_pool(name="x2", bufs=2))
            tm_p = ctx.enter_context(tc.tile_pool(name="tm", bufs=2))
            h3_p = ctx.enter_context(tc.tile_pool(name="h3", bufs=2))
            tl_p = ctx.enter_context(tc.tile_pool(name="tl", bufs=1))
            ps = ctx.enter_context(tc.tile_pool(name="ps", bufs=1,
                                                space="PSUM"))

            weights = singles.tile([128, WF], f32r)
            nc.sync.dma_start(out=weights, in_=wb[:, :])
            wp16 = singles.tile([F_FC2 + 1, F_OUT], f16)
            nc.sync.dma_start(out=wp16, in_=wp16d[:, :])
            b4sb = singles.tile([F_FC2, 1], f32)
            nc.sync.dma_start(out=b4sb, in_=wb4[:, :])

            # double-buffered activation tiles with persistent ones rows
            std_t = [singles.tile([F_STD + 1, NSUB], f32r, name=f"stdb{k}")
                     for k in range(2)]
            h1_t = [singles.tile([C1_H + 1, 3, NSUB], f32r, name=f"h1b{k}")
                    for k in range(2)]
            h2_t = [singles.tile([C2_M + 1, 3, NSUB], f32r, name=f"h2b{k}")
                    for k in range(2)]
            h4_t = [singles.tile([F_FC2 + 1, NSUB], f16, name=f"h4b{k}")
                    for k in range(2)]
            ones_src = _w(weights, "ones")            # [1, 1536] of 1.0
            ones3 = ones_src.rearrange("a (b c) -> a b c", b=3)
            for k in range(2):
                # ones rows via DMA (memset can't write f32r; DMA is exempt
                # from the partition-alignment rule). They persist: the
                # per-subtile sqrt/relu writes only rows 0..N-1.
                nc.sync.dma_start(out=std_t[k][F_STD:F_STD + 1, :],
                                  in_=ones_src[:, 0:NSUB])
                nc.sync.dma_start(out=h1_t[k][C1_H:C1_H + 1, :, :],
                                  in_=ones3)
                nc.sync.dma_start(out=h2_t[k][C2_M:C2_M + 1, :, :],
                                  in_=ones3)
                nc.sync.dma_start(out=h4_t[k][F_FC2:F_FC2 + 1, :],
                                  in_=ones16d[:, :])

            lgall = tl_p.tile([128, nst, 4 * F_OUT], f32, name="lgall")

            xA = [None] * nst
            xB = [None] * nst
            x2A = [None] * nst
            x2B = [None] * nst

            def dma_in(i):
                xA[i] = xa_p.tile([128, NSUB], f32r, name=f"xA{i}", tag="xA")
                nc.sync.dma_start(out=xA[i], in_=sigA[i])
                xB[i] = xb_p.tile([19, NSUB], f32r, name=f"xB{i}", tag="xB")
                nc.sync.dma_start(out=xB[i], in_=sigB[i])

            def squares(i):
                x2A[i] = x2_p.tile([128, NSUB], f32r, name=f"x2A{i}",
                                   tag="x2A")
                nc.gpsimd.tensor_mul(out=x2A[i], in0=xA[i], in1=xA[i])
                x2B[i] = x2_p.tile([19, NSUB], f32r, name=f"x2B{i}", tag="x2B")
                nc.gpsimd.tensor_mul(out=x2B[i], in0=xB[i], in1=xB[i])

            dma_in(0)
            dma_in(1)
            squares(0)

            for q in range(nst + 1):
                if q + 2 < nst:
                    dma_in(q + 2)

                if q < nst:
                    # ---- windowed sums for subtile q ----
                    s_ps = ps.tile([F_STD, NSUB], f32, name=f"s{q}", tag="s")
                    nc.tensor.matmul(s_ps, _w(weights, "sA_a"), xA[q],
                                     start=True, stop=False)
                    nc.tensor.matmul(s_ps, _w(weights, "sA_b"), xB[q],
                                     start=False, stop=True)
                    s2_ps = ps.tile([F_STD, NSUB], f32, name=f"s2{q}",
                                    tag="s2")
                    nc.tensor.matmul(s2_ps, _w(weights, "s2_a"), x2A[q],
                                     start=True, stop=False)
                    nc.tensor.matmul(s2_ps, _w(weights, "s2_b"), x2B[q],
                                     start=False, stop=True)

                    # ---- std = sqrt((s2 - s^2)/9), ones row persists ----
                    t_sb = tm_p.tile([F_STD, NSUB], f32, name=f"t{q}", tag="t")
                    nc.scalar.activation(out=t_sb, in_=s_ps, func=AF.Square)
                    if q + 1 < nst:
                        squares(q + 1)
                    u_sb = tm_p.tile([F_STD, NSUB], f32, name=f"u{q}", tag="u")
                    nc.vector.tensor_sub(out=u_sb, in0=s2_ps, in1=t_sb)
                    nc.scalar.activation(out=std_t[q % 2][0:F_STD, :],
                                         in_=u_sb, func=AF.Sqrt,
                                         scale=1.0 / 9.0)

                if q == 0:
                    continue
                p = q - 1
                B = p % 2
                std_b, h1_b, h2_b, h4_b = (std_t[B], h1_t[B], h2_t[B],
                                           h4_t[B])

                # ---- conv1: 3 chunks into one 3-bank PSUM tile ----
                h1_ps = ps.tile([C1_H, 3, NSUB], f32, name=f"h1p{p}",
                                tag="big1")
                for m in range(3):
                    nc.tensor.matmul(h1_ps[:, m, :], _w(weights, f"c1_{m}"),
                                     std_b)
                nc.scalar.activation(out=h1_b[0:C1_H, :, :], in_=h1_ps,
                                     func=AF.Relu)

                # ---- conv2: 3 single-K matmuls into one 3-bank tile ----
                h2_ps = ps.tile([C2_M, 3, NSUB], f32, name=f"h2p{p}",
                                tag="big2")
                for m in range(3):
                    nc.tensor.matmul(h2_ps[:, m, :], _w(weights, f"c2_{m}"),
                                     h1_b[:, m, :])
                nc.vector.tensor_scalar_max(out=h2_b[0:C2_M, :, :],
                                            in0=h2_ps, scalar1=0.0)

                # ---- fc1 (360 -> 256), bias in k==0 stationary ----
                h3_ps = ps.tile([128, 2, NSUB], f32, name=f"h3p{p}",
                                tag="big1")
                for m in range(2):
                    for k in range(3):
                        nc.tensor.matmul(h3_ps[:, m, :],
                                         _w(weights, f"f1_{k}_{m}"),
                                         h2_b[:, k, :], start=(k == 0),
                                         stop=(k == 2))
                h3_sb = h3_p.tile([128, 2, NSUB], f32r, name=f"h3_{p}",
                                  tag="h3")
                nc.scalar.activation(out=h3_sb, in_=h3_ps, func=AF.Relu)

                # ---- fc2 (256 -> 64) + bias + relu ----
                h4_ps = ps.tile([F_FC2, NSUB], f32, name=f"h4p{p}", tag="big2")
                for k in range(2):
                    nc.tensor.matmul(h4_ps, _w(weights, f"f2_{k}"),
                                     h3_sb[:, k, :], start=(k == 0),
                                     stop=(k == 1))
                nc.vector.tensor_scalar(out=h4_b[0:F_FC2, :], in0=h4_ps,
                                        scalar1=b4sb[:, 0:1], scalar2=0.0,
                                        op0=ALU.add, op1=ALU.max)

                # ---- pamap: logits sample-major, bias via ones row ----
                lg_ps = ps.tile([128, 4 * F_OUT], f32, name=f"lg{p}",
                                tag="big2")
                for j in range(4):
                    nc.tensor.matmul(
                        lg_ps[:, j * F_OUT:(j + 1) * F_OUT],
                        h4_b[:, j * 128:(j + 1) * 128], wp16,
                        start=True, stop=True)
                nc.vector.tensor_copy(out=lgall[:, p, :], in_=lg_ps)

            # ---------- batched log_softmax tail, two halves ----------
            half = nst // 2
            lg4 = lgall.rearrange("a (h i) (j o) -> a h i j o", h=2, o=F_OUT)
            for h in range(2):
                lgh = lg4[:, h]                          # [128, half, 4, 12]
                e = tl_p.tile([128, half, 4, F_OUT], f32, name=f"e{h}",
                              tag="e", bufs=2)
                nc.scalar.activation(out=e, in_=lgh, func=AF.Exp)
                ssum = tl_p.tile([128, half, 4], f32, name=f"ss{h}",
                                 tag="ss", bufs=2)
                nc.vector.tensor_reduce(out=ssum, in_=e,
                                        axis=mybir.AxisListType.X, op=ALU.add)
                lse = tl_p.tile([128, half, 4], f32, name=f"lse{h}",
                                tag="lse", bufs=2)
                nc.scalar.activation(out=lse, in_=ssum, func=AF.Ln)
                lse4 = bass.AP(tensor=lse.tensor, offset=lse.offset,
                               ap=[lse.ap[0], lse.ap[1], lse.ap[2],
                                   [0, F_OUT]])
                ot = tl_p.tile([128, half, 4, F_OUT], f32, name=f"ot{h}",
                               tag="ot", bufs=2)
                nc.gpsimd.tensor_tensor(out=ot, in0=lgh, in1=lse4,
                                        op=ALU.subtract)
                nc.sync.dma_start(out=out_v[h], in_=ot)

    _split_excess_waits(nc)
    return nc


def _get_program(nst):
    global _PROGRAM
    if _PROGRAM is None or _PROGRAM[0] != nst:
        _PROGRAM = (nst, _build_program(nst))
    return _PROGRAM[1]


# ---------------------------------------------------------------------------
# Entry point
# ---------------------------------------------------------------------------
def kernel(signal, conv1_w, conv1_b, conv2_w, conv2_b, fc1_w, fc1_b,
           fc2_w, fc2_b, pamap_w, pamap_b, **_unused):
    from concourse.bass_utils import run_bass_kernel_spmd

    nst = _NST
    b_core = nst * NSUB
    signal = np.asarray(signal, np.float32)
    b_tot = signal.shape[0]
    assert b_tot == b_core * NCORES, (b_tot, b_core)

    blob, bias4, wp16 = _build_blob(np.asarray(conv1_w), np.asarray(conv1_b),
                             np.asarray(conv2_w), np.asarray(conv2_b),
                             np.asarray(fc1_w), np.asarray(fc1_b),
                             np.asarray(fc2_w), np.asarray(fc2_b),
                             np.asarray(pamap_w), np.asarray(pamap_b))

    nc = _get_program(nst)
    in_maps = _prep_core_inputs(signal, blob, bias4, wp16)
    res = run_bass_kernel_spmd(nc, in_maps, core_ids=list(range(NCORES)))
    outs = [res.results[c]["out"] for c in range(NCORES)]
    return np.concatenate(outs, axis=0)


# revision 9
# speedup vs baseline: 1.0094x; 1.0094x over previous
"""Trainium2 Bass kernel for the ConvNet problem.

Per 512-sample subtile (feature-major via host-side transpose):
  windowed sums s=A@x/sqrt(10), s2=A@x^2 (banded matmuls) -> std =
  sqrt((s2-s^2)/9) -> conv1 (dense 120->190 as 3 overlapping 70-row
  chunks) -> conv2 (3 single-K matmuls, 120 outs each) -> fc1 -> fc2 ->
  pamap (weight-streaming, fp16 weights) -> batched log_softmax tail.

All matmul operands are float32r (full-rate PE, fp32 bits).  Biases are
folded into the matmuls via ones-rows appended to the activations, so
every epilogue is a single merged relu over a multi-bank PSUM tile.
Sharding: pure data parallelism, batch split 8 ways across NeuronCores.
"""

import math
import os

import numpy as np

# ---------------------------------------------------------------------------
# Problem constants (hardcoded; kernel.py must be self-contained)
# ---------------------------------------------------------------------------
B_TOTAL, L, C = 131072, 50, 3
WIN = 10
NCORES = 8
B_CORE = B_TOTAL // NCORES          # 16384
NSUB = 512                          # samples per subtile (matmul N)
NST_FULL = B_CORE // NSUB           # 32 subtiles per core
F_IN = L * C                        # 150
F_USE = 147                         # features consumed (x[49,:] unused)
F_STD = 120                         # 40 windows x 3 channels
F_C1 = 190                          # 38 x 5
F_C2 = 360                          # 36 x 10
F_FC2 = 64
F_OUT = 12

# conv1 emitted as 3 overlapping 70-row chunks; conv2 out chunks of 120
C1_LO = (0, 60, 120)
C1_H = 70                           # rows per conv1 chunk (+1 ones row)
C2_M = 120                          # conv2 outs per chunk (3 chunks = 360)

# debug knob (harness never sets this; default = full problem)
_NST = int(os.environ.get("ATRN_NST", str(NST_FULL)))


# ---------------------------------------------------------------------------
# Tile drain patch: walrus in this container rejects >2 sem waits on a
# CTRL-class (Drain) instruction.  Spread the end-of-kernel global-clock waits
# across per-proc SP nops (one sem each) before an unadorned drain.
# ---------------------------------------------------------------------------
def _install_drain_patch():
    import concourse.tile as tile
    from concourse.tile_scheduler import N_PROCS
    from concourse.vector_clock import ScopedClock, VectorClock

    if getattr(tile.TileContext, "_drain_patch_installed", False):
        return

    def _patched_drain_and_barrier(self, tick_clock, wait_clock):
        nc = self.nc
        gc = tick_clock.global_clock
        for p in range(N_PROCS):
            if gc[p] <= 0:
                continue
            v = [0] * N_PROCS
            v[p] = gc[p]
            nop = nc.sync.nop()
            wait_clock.add_sem_waits(nop.ins, ScopedClock({None: VectorClock(v)}))
        nc.sync.drain()
        nc.all_engine_barrier()
        assert self.sems is not None
        popped = nc._tile_sem_poison_stack.pop()
        assert popped is self._sem_poison
        nc.clear_and_free_semaphores(list(self.sems.allocated().values()))
        nc.all_engine_barrier()

    tile.TileContext._drain_and_barrier = _patched_drain_and_barrier
    tile.TileContext._drain_patch_installed = True


def _split_excess_waits(nc):
    """Hoist excess sem waits onto same-engine nops inserted just before."""
    from concourse import mybir

    ctr = 0
    for f in nc.m.functions:
        for blk in f.blocks:
            il = blk.instructions
            i = 0
            while i < len(il):
                ins = il[i]
                si = ins.sync_info
                cap = 1  # this walrus build rejects >1 sem wait per engine op
                if si is not None and len(si.on_wait) > cap:
                    waits = list(si.on_wait)
                    extra, keep = waits[:-cap], waits[-cap:]
                    for w in extra:
                        ctr += 1
                        nop = mybir.InstNoOp(name=f"waitsplit-{ctr}",
                                             ins=[], outs=[])
                        nop.engine = ins.engine
                        nop.sync_info = type(si)(on_wait=[w], on_update=[])
                        nc.register_instruction(nop, overwrite=True)
                        il.insert(i, nop)
                        i += 1
                    ins.sync_info = type(si)(on_wait=keep,
                                             on_update=list(si.on_update))
                i += 1


# ---------------------------------------------------------------------------
# Host-side weight preprocessing
# ---------------------------------------------------------------------------
class _BlobLayout:
    def __init__(self):
        self.cols = 0
        self.slots = {}

    def add(self, name, rows, cols):
        self.slots[name] = (self.cols, rows, cols)
        self.cols += cols
        return self.slots[name]


_LAY = _BlobLayout()
_LAY.add("sA_a", 128, F_STD)
_LAY.add("sA_b", 19, F_STD)
_LAY.add("s2_a", 128, F_STD)
_LAY.add("s2_b", 19, F_STD)
for m in range(3):
    _LAY.add(f"c1_{m}", F_STD + 1, C1_H)         # +bias row (std ones row)
for m in range(3):
    _LAY.add(f"c2_{m}", C1_H + 1, C2_M)          # +bias row (h1 ones row)
for k in range(3):
    for m in range(2):
        _LAY.add(f"f1_{k}_{m}", C2_M + 1, 128)   # +bias row (k==0 only)
for k in range(2):
    _LAY.add(f"f2_{k}", 128, F_FC2)
_LAY.add("ones", 1, 3 * NSUB)
WF = _LAY.cols


def _build_blob(conv1_w, conv1_b, conv2_w, conv2_b, fc1_w, fc1_b, fc2_w, fc2_b,
                pamap_w, pamap_b):
    blob = np.zeros((128, WF), np.float32)
    bias4 = np.zeros((F_FC2, 1), np.float32)
    bias4[:, 0] = fc2_b.astype(np.float32)

    def put(name, arr):
        off, rows, cols = _LAY.slots[name]
        assert arr.shape == (rows, cols), (name, arr.shape, (rows, cols))
        blob[:rows, off:off + cols] = arr

    # windowed sums: s[3w+c] = (1/sqrt(10)) * sum_k x[3(w+k)+c]
    A = np.zeros((F_USE, F_STD), np.float32)
    for m in range(F_STD):
        w, c = divmod(m, 3)
        for k in range(WIN):
            A[3 * (w + k) + c, m] = 1.0
    put("ones", np.ones((1, 3 * NSUB), np.float32))
    put("sA_a", A[:128] / math.sqrt(10.0))
    put("sA_b", A[128:] / math.sqrt(10.0))
    put("s2_a", A[:128])
    put("s2_b", A[128:])

    # conv1 dense [120 -> 190] + bias row; 3 overlapping 70-row out chunks
    M1 = np.zeros((F_STD + 1, F_C1), np.float32)
    for t in range(38):
        for o in range(5):
            M1[F_STD, 5 * t + o] = conv1_b[o]
            for k in range(3):
                for i in range(3):
                    M1[3 * (t + k) + i, 5 * t + o] = conv1_w[o, i, k]
    for m in range(3):
        put(f"c1_{m}", M1[:, C1_LO[m]:C1_LO[m] + C1_H])

    # conv2 dense [190 -> 360] + bias row; chunk m reads h1 rows
    # C1_LO[m]..C1_LO[m]+69 and writes outs 120m..120m+119
    M2 = np.zeros((F_C1, F_C2), np.float32)
    for t in range(36):
        for o in range(10):
            for k in range(3):
                for i in range(5):
                    M2[5 * (t + k) + i, 10 * t + o] = conv2_w[o, i, k]
    b2 = np.zeros(F_C2, np.float32)
    for t in range(36):
        for o in range(10):
            b2[10 * t + o] = conv2_b[o]
    for m in range(3):
        sub = np.zeros((C1_H + 1, C2_M), np.float32)
        sub[:C1_H] = M2[C1_LO[m]:C1_LO[m] + C1_H, C2_M * m:C2_M * (m + 1)]
        sub[C1_H] = b2[C2_M * m:C2_M * (m + 1)]
        put(f"c2_{m}", sub)

    # fc1 [360 -> 256]; K chunks = conv2 out chunks (120 each + ones row)
    F1 = fc1_w.T.astype(np.float32)          # [360, 256]
    for k in range(3):
        for m in range(2):
            sub = np.zeros((C2_M + 1, 128), np.float32)
            sub[:C2_M] = F1[C2_M * k:C2_M * (k + 1), 128 * m:128 * (m + 1)]
            if k == 0:
                sub[C2_M] = fc1_b[128 * m:128 * (m + 1)]
            put(f"f1_{k}_{m}", sub)
    F2 = fc2_w.T.astype(np.float32)          # [256, 64]
    for k in range(2):
        put(f"f2_{k}", F2[k * 128:(k + 1) * 128])

    # pamap weights + bias row (h4 ones row), fp16
    wp16 = np.zeros((F_FC2 + 1, F_OUT), np.float16)
    wp16[:F_FC2] = pamap_w.T.astype(np.float16)
    wp16[F_FC2] = pamap_b.astype(np.float16)
    return blob, bias4, wp16


def _prep_core_inputs(signal, blob, bias4, wp16):
    """Split + feature-major transpose the signal; one input map per core."""
    nst = _NST
    b_core = nst * NSUB
    sig = np.asarray(signal, np.float32).reshape(-1, F_IN)
    maps = []
    for c in range(NCORES):
        s = sig[c * b_core:(c + 1) * b_core]            # [b_core, 150]
        s3 = s.reshape(nst, NSUB, F_IN)                 # [nst, 512, 150]
        sigA = np.ascontiguousarray(s3[:, :, 0:128].transpose(0, 2, 1))
        sigB = np.ascontiguousarray(s3[:, :, 128:F_USE].transpose(0, 2, 1))
        maps.append({"sigA": sigA, "sigB": sigB, "wb": blob,
                     "wb4": bias4, "wp16": wp16,
                     "ones16": np.ones((1, NSUB), np.float16)})
    return maps


# ---------------------------------------------------------------------------
# Bass program
# ---------------------------------------------------------------------------
_PROGRAM = None


def _build_program(nst):
    import contextlib

    import concourse.bass as bass
    import concourse.tile as tile
    from concourse import mybir

    _install_drain_patch()
    f32 = mybir.dt.float32
    f32r = mybir.dt.float32r
    f16 = mybir.dt.float16
    AF = mybir.ActivationFunctionType
    ALU = mybir.AluOpType

    nc = bass.Bass("TRN2", target_bir_lowering=False, debug=False,
                   num_devices=NCORES)
    sigA = nc.dram_tensor("sigA", [nst, 128, NSUB], f32r, kind="ExternalInput")
    sigB = nc.dram_tensor("sigB", [nst, 19, NSUB], f32r, kind="ExternalInput")
    wb = nc.dram_tensor("wb", [128, WF], f32r, kind="ExternalInput")
    wp16d = nc.dram_tensor("wp16", [F_FC2 + 1, F_OUT], f16,
                           kind="ExternalInput")
    wb4 = nc.dram_tensor("wb4", [F_FC2, 1], f32, kind="ExternalInput")
    ones16d = nc.dram_tensor("ones16", [1, NSUB], f16, kind="ExternalInput")
    out = nc.dram_tensor("out", [nst * NSUB, F_OUT], f32,
                         kind="ExternalOutput")
    # sample index = q*512 + j*128 + p  ->  [half][p][i][j][o]
    out_v = out.rearrange("(h i j p) o -> h p i j o", h=2, j=4, p=128)

    def _w(weights, name):
        off, rows, cols = _LAY.slots[name]
        return weights[0:rows, off:off + cols]

    with tile.TileContext(nc) as tc:
        with contextlib.ExitStack() as ctx:
            singles = ctx.enter_context(tc.tile_pool(name="singles", bufs=1))
            xa_p = ctx.enter_context(tc.tile_pool(name="xa", bufs=3))
            xb_p = ctx.enter_context(tc.tile_pool(name="xb", bufs=3))
            x2_p = ctx.enter_context(tc.tile_pool(name="x2", bufs=2))
            tm_p = ctx.enter_context(tc.tile_pool(name="tm", bufs=2))
            h3_p = ctx.enter_context(tc.tile_pool(name="h3", bufs=2))
            tl_p = ctx.enter_context(tc.tile_pool(name="tl", bufs=1))
            ps = ctx.enter_context(tc.tile_pool(name="ps", bufs=1,
                                                space="PSUM"))

            weights = singles.tile([128, WF], f32r)
            nc.sync.dma_start(out=weights, in_=wb[:, :])
            wp16 = singles.tile([F_FC2 + 1, F_OUT], f16)
            nc.sync.dma_start(out=wp16, in_=wp16d[:, :])
            b4sb = singles.tile([F_FC2, 1], f32)
            nc.sync.dma_start(out=b4sb, in_=wb4[:, :])

            # double-buffered activation tiles with persistent ones rows
            std_t = [singles.tile([F_STD + 1, NSUB], f32r, name=f"stdb{k}")
                     for k in range(2)]
            h1_t = [singles.tile([C1_H + 1, 3, NSUB], f32r, name=f"h1b{k}")
                    for k in range(2)]
            h2_t = [singles.tile([C2_M + 1, 3, NSUB], f32r, name=f"h2b{k}")
                    for k in range(2)]
            h4_t = [singles.tile([F_FC2 + 1, NSUB], f16, name=f"h4b{k}")
                    for k in range(2)]
            ones_src = _w(weights, "ones")            # [1, 1536] of 1.0
            ones3 = ones_src.rearrange("a (b c) -> a b c", b=3)
            for k in range(2):
                # ones rows via DMA (memset can't write f32r; DMA is exempt
                # from the partition-alignment rule). They persist: the
                # per-subtile sqrt/relu writes only rows 0..N-1.
                nc.sync.dma_start(out=std_t[k][F_STD:F_STD + 1, :],
                                  in_=ones_src[:, 0:NSUB])
                nc.sync.dma_start(out=h1_t[k][C1_H:C1_H + 1, :, :],
                                  in_=ones3)
                nc.sync.dma_start(out=h2_t[k][C2_M:C2_M + 1, :, :],
                                  in_=ones3)
                nc.sync.dma_start(out=h4_t[k][F_FC2:F_FC2 + 1, :],
                                  in_=ones16d[:, :])

            lgall = tl_p.tile([128, nst, 4 * F_OUT], f32, name="lgall")

            xA = [None] * nst
            xB = [None] * nst
            x2A = [None] * nst
            x2B = [None] * nst

            def dma_in(i):
                xA[i] = xa_p.tile([128, NSUB], f32r, name=f"xA{i}", tag="xA")
                nc.sync.dma_start(out=xA[i], in_=sigA[i])
                xB[i] = xb_p.tile([19, NSUB], f32r, name=f"xB{i}", tag="xB")
                nc.sync.dma_start(out=xB[i], in_=sigB[i])

            def squares(i):
                x2A[i] = x2_p.tile([128, NSUB], f32r, name=f"x2A{i}",
                                   tag="x2A")
                nc.gpsimd.tensor_mul(out=x2A[i], in0=xA[i], in1=xA[i])
                x2B[i] = x2_p.tile([19, NSUB], f32r, name=f"x2B{i}", tag="x2B")
                nc.gpsimd.tensor_mul(out=x2B[i], in0=xB[i], in1=xB[i])

            dma_in(0)
            dma_in(1)
            dma_in(2)
            squares(0)

            # Skewed software pipeline: window q runs conv2(q-1) -> s/s2(q+1)
            # -> fc1(q-1) -> conv1(q) -> pamap(q-2) -> fc2(q-1) on PE so each
            # matmul's inputs were epilogued in an earlier window slot.
            # PSUM tags: s(1) s2(1) h1(3) chain h2->h3->lg->h4 (3) = 8 banks.
            std_s = [None] * nst
            h1_ps_s = [None] * nst
            h3_sb_s = [None] * nst

            def emit_s(i):
                s_ps = ps.tile([F_STD, NSUB], f32, name=f"s{i}", tag="s")
                nc.tensor.matmul(s_ps, _w(weights, "sA_a"), xA[i],
                                 start=True, stop=False)
                nc.tensor.matmul(s_ps, _w(weights, "sA_b"), xB[i],
                                 start=False, stop=True)
                s2_ps = ps.tile([F_STD, NSUB], f32, name=f"s2{i}", tag="s2")
                nc.tensor.matmul(s2_ps, _w(weights, "s2_a"), x2A[i],
                                 start=True, stop=False)
                nc.tensor.matmul(s2_ps, _w(weights, "s2_b"), x2B[i],
                                 start=False, stop=True)
                t_sb = tm_p.tile([F_STD, NSUB], f32, name=f"t{i}", tag="t")
                nc.scalar.activation(out=t_sb, in_=s_ps, func=AF.Square)
                u_sb = tm_p.tile([F_STD, NSUB], f32, name=f"u{i}", tag="u")
                nc.vector.tensor_sub(out=u_sb, in0=s2_ps, in1=t_sb)
                std_s[i] = std_t[i % 2]
                nc.scalar.activation(out=std_s[i][0:F_STD, :], in_=u_sb,
                                     func=AF.Sqrt, scale=1.0 / 9.0)

            if nst > 1:
                squares(1)
            emit_s(0)

            for q in range(nst + 2):
                if q + 3 < nst:
                    dma_in(q + 3)
                p = q - 1          # conv2/fc1/fc2 stage
                r = q - 2          # pamap stage
                i = q + 1          # windowed-sum stage

                # ---- conv2(p): 3 single-K matmuls -> 3-bank tile ----
                if 0 <= p < nst:
                    h2_ps = ps.tile([C2_M, 3, NSUB], f32, name=f"h2p{p}",
                                    tag="chain")
                    h1_b = h1_t[p % 2]
                    for m in range(3):
                        nc.tensor.matmul(h2_ps[:, m, :],
                                         _w(weights, f"c2_{m}"),
                                         h1_b[:, m, :])
                    h2_b = h2_t[p % 2]
                    nc.scalar.activation(out=h2_b[0:C2_M, 0:2, :],
                                         in_=h2_ps[:, 0:2, :], func=AF.Relu)
                    nc.vector.tensor_scalar_max(out=h2_b[0:C2_M, 2, :],
                                                in0=h2_ps[:, 2, :],
                                                scalar1=0.0)

                # ---- s/s2(i) + std chain; squares for i+1 ----
                if i < nst:
                    emit_s(i)
                if i + 1 < nst:
                    squares(i + 1)

                # ---- fc1(p) ----
                if 0 <= p < nst:
                    h3_ps = ps.tile([128, 2, NSUB], f32, name=f"h3p{p}",
                                    tag="chain")
                    h2_b = h2_t[p % 2]
                    for m in range(2):
                        for k in range(3):
                            nc.tensor.matmul(h3_ps[:, m, :],
                                             _w(weights, f"f1_{k}_{m}"),
                                             h2_b[:, k, :], start=(k == 0),
                                             stop=(k == 2))
                    h3_sb = h3_p.tile([128, 2, NSUB], f32r, name=f"h3_{p}",
                                      tag="h3")
                    nc.scalar.activation(out=h3_sb[:, 0, :],
                                         in_=h3_ps[:, 0, :], func=AF.Relu)
                    nc.vector.tensor_scalar_max(out=h3_sb[:, 1, :],
                                                in0=h3_ps[:, 1, :],
                                                scalar1=0.0)
                    h3_sb_s[p] = h3_sb

                # ---- conv1(q) ----
                if q < nst:
                    h1_ps = ps.tile([C1_H, 3, NSUB], f32, name=f"h1p{q}",
                                    tag="h1")
                    for m in range(3):
                        nc.tensor.matmul(h1_ps[:, m, :],
                                         _w(weights, f"c1_{m}"), std_s[q])
                    h1_b = h1_t[q % 2]
                    nc.scalar.activation(out=h1_b[0:C1_H, 0:2, :],
                                         in_=h1_ps[:, 0:2, :], func=AF.Relu)
                    nc.vector.tensor_scalar_max(out=h1_b[0:C1_H, 2, :],
                                                in0=h1_ps[:, 2, :],
                                                scalar1=0.0)

                # ---- pamap(r): stationary h4, fp16 streaming weights ----
                if 0 <= r < nst:
                    lg_ps = ps.tile([128, 4 * F_OUT], f32, name=f"lg{r}",
                                    tag="chain")
                    h4_b = h4_t[r % 2]
                    for j in range(4):
                        nc.tensor.matmul(
                            lg_ps[:, j * F_OUT:(j + 1) * F_OUT],
                            h4_b[:, j * 128:(j + 1) * 128], wp16,
                            start=True, stop=True)
                    nc.vector.tensor_copy(out=lgall[:, r, :], in_=lg_ps)

                # ---- fc2(p) + bias + relu ----
                if 0 <= p < nst:
                    h4_ps = ps.tile([F_FC2, NSUB], f32, name=f"h4p{p}",
                                    tag="chain")
                    for k in range(2):
                        nc.tensor.matmul(h4_ps, _w(weights, f"f2_{k}"),
                                         h3_sb_s[p][:, k, :], start=(k == 0),
                                         stop=(k == 1))
                    nc.vector.tensor_scalar(out=h4_t[p % 2][0:F_FC2, :],
                                            in0=h4_ps,
                                            scalar1=b4sb[:, 0:1], scalar2=0.0,
                                            op0=ALU.add, op1=ALU.max)

            # ---------- batched log_softmax tail, two halves ----------
            half = nst // 2
            lg4 = lgall.rearrange("a (h i) (j o) -> a h i j o", h=2, o=F_OUT)
            for h in range(2):
                lgh = lg4[:, h]                          # [128, half, 4, 12]
                e = tl_p.tile([128, half, 4, F_OUT], f32, name=f"e{h}",
                              tag="e", bufs=2)
                nc.scalar.activation(out=e, in_=lgh, func=AF.Exp)
                ssum = tl_p.tile([128, half, 4], f32, name=f"ss{h}",
                                 tag="ss", bufs=2)
                nc.vector.tensor_reduce(out=ssum, in_=e,
                                        axis=mybir.AxisListType.X, op=ALU.add)
                lse = tl_p.tile([128, half, 4], f32, name=f"lse{h}",
                                tag="lse", bufs=2)
                nc.scalar.activation(out=lse, in_=ssum, func=AF.Ln)
                lse4 = bass.AP(tensor=lse.tensor, offset=lse.offset,
                               ap=[lse.ap[0], lse.ap[1], lse.ap[2],
                                   [0, F_OUT]])
                ot = tl_p.tile([128, half, 4, F_OUT], f32, name=f"ot{h}",
                               tag="ot", bufs=2)
                nc.gpsimd.tensor_tensor(out=ot, in0=lgh, in1=lse4,
                                        op=ALU.subtract)
                nc.sync.dma_start(out=out_v[h], in_=ot)

    _split_excess_waits(nc)
    return nc


def _get_program(nst):
    global _PROGRAM
    if _PROGRAM is None or _PROGRAM[0] != nst:
        _PROGRAM = (nst, _build_program(nst))
    return _PROGRAM[1]


# ---------------------------------------------------------------------------
# Entry point
# ---------------------------------------------------------------------------
def kernel(signal, conv1_w, conv1_b, conv2_w, conv2_b, fc1_w, fc1_b,
           fc2_w, fc2_b, pamap_w, pamap_b, **_unused):
    from concourse.bass_utils import run_bass_kernel_spmd

    nst = _NST
    b_core = nst * NSUB
    signal = np.asarray(signal, np.float32)
    b_tot = signal.shape[0]
    assert b_tot == b_core * NCORES, (b_tot, b_core)

    blob, bias4, wp16 = _build_blob(np.asarray(conv1_w), np.asarray(conv1_b),
                             np.asarray(conv2_w), np.asarray(conv2_b),
                             np.asarray(fc1_w), np.asarray(fc1_b),
                             np.asarray(fc2_w), np.asarray(fc2_b),
                             np.asarray(pamap_w), np.asarray(pamap_b))

    nc = _get_program(nst)
    in_maps = _prep_core_inputs(signal, blob, bias4, wp16)
    res = run_bass_kernel_spmd(nc, in_maps, core_ids=list(range(NCORES)))
    outs = [res.results[c]["out"] for c in range(NCORES)]
    return np.concatenate(outs, axis=0)
